# revision 36
# baseline (speedup 1.0000x reference)
"""Trainium2 Bass kernel for nn_AConnect (A-Connect dense MLP forward).

Computes  Z[b,o] = sum_i X[b,i] * W[i,o] * Werr[b,i,o] + bias[o] * Berr[b,o]
with B=128, ROW=OUT=1024, f32 inputs/outputs.

Strategy (pure data parallel over batch, 8 NeuronCores, 16 batches/core):
  - Werr dominates traffic: memory-bound kernel. Host casts Werr/W/X to
    bf16 (the X*W*Werr product accumulates in f32 PSUM; measured rel err
    ~3e-3 vs the f32 reference), halving HBM bytes to 32 MB/core. The
    dual-ring DMA stream below measures 315-406 GB/s per core depending
    on how the HBM-stack partner core's run overlaps.
  - Werr[b] arrives as [128p x (8c x 1024o)] with partition p holding 8
    contiguous rows (i = 8p + c), so each DMA is fully-contiguous 16 KB
    runs. Each batch is split into two 1 MB DMAs alternated across the
    two HWDGE rings (sync/scalar).
  - VectorE computes Q = W .* Werr[b] in place (bf16 tensor_tensor in 2x
    mode, issued as [128, 2048] slices of flattened 2D views — wider ops
    amortize the per-op overhead, but only clean 2D APs keep 2x mode).
  - TensorE: batches are processed in pairs; the 4 output rows of a pair
    (2 batches x 2 output halves) map to the 4 PE column groups
    (tile_position (0, 32j), out partition 32j), so 4 matmuls with
    different stationaries run concurrently in the array instead of
    serializing on per-matmul weight loads. Each group accumulates its 8
    contraction chunks into its own PSUM bank.
  - ScalarE copies the 4 PSUM rows to an SBUF stage tile; one SWDGE DMA
    with accum_op=add scatters them onto the output DRAM, which was
    preloaded with the host-precomputed f32 bias*Berr rows (the bias
    path stays full f32). The last pair instead fuses the bias add into
    DVE PSUM->SBUF moves and plain-writes via HWDGE, keeping the SWDGE
    accum latency off the critical tail.

The i-permutation (partition p, slot c <-> row 8p+c) is applied to X on
the host; the contraction is order-agnostic so W/Werr/X just need the
same layout.
"""

import numpy as np

B, ROW, OUT = 128, 1024, 1024
NCORES = 8
NB = B // NCORES          # 16 batches per core
P = 128                   # partitions
NCH = ROW // P            # 8 contraction chunks (slot c on partition p = row 8p+c)
HALF = 512                # PSUM bank limit for matmul output (f32)

_CACHE = {}


def _build():
    if "nc" in _CACHE:
        return _CACHE["nc"]
    from concourse import bacc, mybir, tile

    f32 = mybir.dt.float32
    bf16 = mybir.dt.bfloat16

    nc = bacc.Bacc("TRN2", target_bir_lowering=False, debug=False,
                   num_devices=NCORES)
    w_d = nc.declare_dram_parameter("w", [ROW, OUT], bf16, isOutput=False)
    xt_d = nc.declare_dram_parameter("xt", [P, NCH, NB], bf16, isOutput=False)
    bb_d = nc.declare_dram_parameter("bb", [NB, OUT], f32, isOutput=False)
    we_d = nc.declare_dram_parameter("werr", [NB, ROW, OUT], bf16,
                                     isOutput=False)
    out_d = nc.declare_dram_parameter("out", [NB, OUT], f32, isOutput=True)

    with tile.TileContext(nc) as tc:
        with tc.tile_pool(name="const", bufs=1) as cpool, \
             tc.tile_pool(name="werr", bufs=8) as wepool, \
             tc.tile_pool(name="stage", bufs=3) as spool, \
             tc.tile_pool(name="ps", bufs=2, space="PSUM") as pspool:

            w_sb = cpool.tile([P, NCH, OUT], bf16, tag="w_sb")
            xt_sb = cpool.tile([P, NCH, NB], bf16, tag="xt_sb")

            # Preload bias*Berr rows into the output; per-batch results are
            # DMA-accumulated on top. The last pair instead adds its bias
            # rows on-chip (from bbstage) and plain-writes, avoiding the
            # SWDGE accum latency on the critical tail.
            nc.gpsimd.dma_start(out=out_d[:], in_=bb_d[:])
            nc.gpsimd.dma_start(out=xt_sb[:], in_=xt_d[:])
            bbstage = cpool.tile([P, HALF], f32, tag="bbstage")
            nc.vector.memset(bbstage[:], 0.0)
            nc.gpsimd.dma_start(
                out=bbstage[0:P:32, :],
                in_=bb_d[NB - 2:NB].rearrange("b (h o) -> (b h) o", h=2))

            CH2 = NCH // 2
            w_src = w_d[:].rearrange("(p c) o -> p c o", c=NCH)
            for pair in range(NB // 2):
                b0 = 2 * pair
                wes = []
                last = pair == NB // 2 - 1
                for b in (b0, b0 + 1):
                    we = wepool.tile([P, NCH, OUT], bf16, tag="we")
                    src = we_d[b].rearrange("(p c) o -> p c o", c=NCH)
                    if last:
                        # quarter-DMAs on the final pair, with the very
                        # last quarter split per chunk so chunk 6's mult
                        # and matmul round finish before the last byte
                        for q in range(3):
                            ring = nc.sync if q % 2 == 0 else nc.scalar
                            ring.dma_start(out=we[:, 2 * q:2 * q + 2],
                                           in_=src[:, 2 * q:2 * q + 2])
                        nc.scalar.dma_start(out=we[:, 6:7], in_=src[:, 6:7])
                        nc.sync.dma_start(out=we[:, 7:8], in_=src[:, 7:8])
                    else:
                        # two 1 MB halves on the two HWDGE rings
                        nc.sync.dma_start(out=we[:, 0:CH2], in_=src[:, 0:CH2])
                        nc.scalar.dma_start(out=we[:, CH2:NCH],
                                            in_=src[:, CH2:NCH])
                    wes.append(we)
                if pair == 0:
                    # W (replicated, needed by the first mult) rides both
                    # rings right behind pair 0's Werr halves; partition p
                    # holds rows 8p..8p+7, fully contiguous runs.
                    nc.sync.dma_start(out=w_sb[:, 0:CH2], in_=w_src[:, 0:CH2])
                    nc.scalar.dma_start(out=w_sb[:, CH2:NCH],
                                        in_=w_src[:, CH2:NCH])

                if last:
                    # all 4 rows share ONE PSUM bank: the single global
                    # start=True clears the bank's has_written bits once;
                    # each region's first matmul then overwrites, later
                    # ones accumulate (per-element semantics) — so the
                    # whole epilogue collapses to one [128,512] DVE add.
                    psl = pspool.tile([P, HALF], f32, tag="ps0",
                                      name="ps_last")
                    # data-wise a no-op (the first matmul's start=True
                    # overwrites), but it initializes the bank for the
                    # simulator's memory tracking and the full-width add
                    nc.vector.memset(psl[:], 0.0)
                    pss = [psl] * 4
                else:
                    pss = [pspool.tile([P, HALF], f32, tag=f"ps{j}",
                                       name=f"ps{j}_{pair}")
                           for j in range(4)]
                stage = spool.tile([P, HALF], f32, tag="stage")

                # flattened 2D views so wider mults keep the DVE 2x mode;
                # the last pair's chunks 6/7 are multiplied per chunk to
                # match its per-chunk tail DMAs
                w2 = w_sb[:].rearrange("p c o -> p (c o)")
                slices = [(0, 2), (2, 4), (4, 6)] + \
                    ([(6, 7), (7, 8)] if last else [(6, 8)])
                for bb_i in range(2):
                    we2 = wes[bb_i][:].rearrange("p c o -> p (c o)")
                    for c0, c1 in slices:
                        sl_ = slice(c0 * OUT, c1 * OUT)
                        nc.vector.tensor_mul(we2[:, sl_], we2[:, sl_],
                                             w2[:, sl_])

                # 4 column groups: j = 2*(b-b0) + half, out partition 32j,
                # one PSUM bank per group
                for c in range(NCH):
                    for j in range(4):
                        bb_i, h = divmod(j, 2)
                        nc.tensor.matmul(
                            pss[j][32 * j:32 * j + 1, :],
                            xt_sb[:, c, b0 + bb_i:b0 + bb_i + 1],
                            wes[bb_i][:, c, h * HALF:(h + 1) * HALF],
                            start=(c == 0 and (not last or j == 0)),
                            stop=(c == NCH - 1 and (not last or j == 3)),
                            skip_group_check=last,
                            tile_position=(0, 32 * j))

                if last:
                    # one full-width add fuses bias + PSUM->SBUF for all 4
                    # rows at once (garbage partitions are never read), then
                    # plain HWDGE writes — no SWDGE accum on the tail
                    nc.vector.tensor_add(stage[:], pss[0][:], bbstage[:])
                    nc.sync.dma_start(
                        out=out_d[b0:b0 + 2].rearrange("b (h o) -> (b h) o",
                                                       h=2),
                        in_=stage[0:128:32, :])
                else:
                    for j in range(4):
                        nc.scalar.copy(stage[32 * j:32 * j + 1, :],
                                       pss[j][32 * j:32 * j + 1, :])
                    # scatter rows {0,32,64,96} onto out[b0:b0+2] with +=
                    nc.gpsimd.dma_start(
                        out=out_d[b0:b0 + 2].rearrange("b (h o) -> (b h) o",
                                                       h=2),
                        in_=stage[0:128:32, :],
                        accum_op=mybir.AluOpType.add)

    nc.compile()
    _CACHE["nc"] = nc
    return nc


def _in_maps(X, W, bias, Werr, Berr):
    import ml_dtypes
    bf16 = ml_dtypes.bfloat16
    X = np.asarray(X, dtype=np.float32)
    W16 = np.ascontiguousarray(np.asarray(W, dtype=np.float32).astype(bf16))
    Werr = np.asarray(Werr, dtype=np.float32)
    BB = np.asarray(bias, dtype=np.float32)[None, :] * \
        np.asarray(Berr, dtype=np.float32)
    maps = []
    for i in range(NCORES):
        sl = slice(i * NB, (i + 1) * NB)
        # xt[p, c, b] = X[b, 8p + c]
        xt = np.ascontiguousarray(
            X[sl].reshape(NB, P, NCH).transpose(1, 2, 0).astype(bf16))
        maps.append({
            "w": W16,
            "xt": xt,
            "bb": np.ascontiguousarray(BB[sl]),
            "werr": np.ascontiguousarray(Werr[sl].astype(bf16)),
        })
    return maps


def kernel(X, W, bias, Werr, Berr):
    import time
    from concourse.bass_utils import run_bass_kernel_spmd
    nc = _build()
    maps = _in_maps(X, W, bias, Werr, Berr)
    # The device pool occasionally throws a transient
    # NRT_EXEC_UNIT_UNRECOVERABLE right after a previous heavy run;
    # it self-recovers within a minute.
    for attempt in range(3):
        try:
            res = run_bass_kernel_spmd(nc, maps, list(range(NCORES)))
            break
        except Exception:
            if attempt == 2:
                raise
            time.sleep(45)
    return np.concatenate([res.results[i]["out"] for i in range(NCORES)],
                          axis=0)


def kernel_profiled(X, W, bias, Werr, Berr, tmpdir=None):
    """Like kernel() but with NTFF tracing; returns (output, exec_time_ns).
    Caller must have installed the axon NTFF profile hook."""
    from concourse.bass_utils import run_bass_kernel_spmd
    nc = _build()
    res = run_bass_kernel_spmd(nc, _in_maps(X, W, bias, Werr, Berr),
                               list(range(NCORES)), trace=True, tmpdir=tmpdir)
    out = np.concatenate([res.results[i]["out"] for i in range(NCORES)],
                         axis=0)
    return out, res.exec_time_ns
